# revision 16
# baseline (speedup 1.0000x reference)
"""Double-centering kernel for Trainium2 (Bass/Tile), 8-core data parallel.

Computes T = -0.5 * (D - row_mean - col_mean + glob_mean) for
D: [256, 512, 512] f32, sharding the batch dim across 8 NeuronCores
(32 matrices per core, no cross-core communication).

The kernel runs in fp16 end-to-end (HBM traffic halves vs f32; the
centering error stays ~1e-3 relative, far inside tolerance) using the
sequential-centering identity:

    csc0[j] = colsum[j] / 1024            (PE: ones/1024 matmuls, PSUM accum)
    w       = -0.5*D + csc0               (DVE stt, accum_out -> rowsum(w))
    T       = w - rowsum(w)/512           (ACT Identity-bias / DVE ts)

The second step's row-accumulator absorbs the global-mean term exactly,
so no separate global sum is needed.

Per-core layout: tiles of 1/2/4 matrices; each [512, 512] matrix is a
[128, 2048] fp16 block (partition p holds its rows 4p..4p+3). Every DMA
moves one matrix (0.5 MiB contiguous); tile sizes taper
(singles -> pairs -> quads -> pairs -> singles) so the pipeline fills
and drains fast while the steady state amortizes csc/PSUM work.

Engine balance per core (ACT/DVE ~100us each pace the 96us DMA union):
  SP:     loads (HWDGE)
  PE:     csc0 = (ones/1024)^T @ D chunks, 4-chunk PSUM accumulation
  ACT:    csc0 PSUM->SBUF fp16 copy (per tile); a' = -a/512;
          pass B Identity-bias on most chunks
  DVE:    pass A stt (w = -0.5*D + csc0, accum rowsums; stt has no fast
          DVE mode so this is a ~98us floor); pass B ts on late tiles
          (it idles during drain otherwise)
  GPSIMD: stores only (SWDGE; its tensor ops are slow and poison DVE
          via the shared SBUF port -- never compute here)
"""

from contextlib import ExitStack

import numpy as np

import concourse.bacc as bacc
import concourse.tile as tile
from concourse import mybir
from concourse.bass_utils import run_bass_kernel_spmd

N_CORES = 8
B = 256
N = 512
B_LOC = B // N_CORES  # 32 matrices per core
P = 128
CHUNKS = N // P  # 4 row-chunks per matrix
FREE = CHUNKS * N  # 2048 elems per partition per matrix
MAXSZ = 4  # largest tile, in matrices

# Tile taper: matrices per tile, summing to B_LOC.
TILE_SIZES = [1, 1, 1, 1, 2, 4, 4, 4, 4, 4, 2, 1, 1, 1, 1]
assert sum(TILE_SIZES) == B_LOC
# Pass B: number of trailing chunks per tile run on DVE ts (rest on ACT).
# Fill/drain tiles go fully to DVE (it idles there); mid-stream quads give
# DVE 2 of 16 chunks so ACT's per-tile backlog stops compounding.
PASS_B_DVE_CHUNKS = [4, 4, 4, 4, 2, 2, 2, 2, 2, 2, 8, 4, 4, 4, 4]

_COMPILED = None
LAST_RESULTS = None  # BassKernelResults of the most recent run (for test harness)


def _build():
    nc = bacc.Bacc("TRN2", target_bir_lowering=False, debug=False)
    f16 = mybir.dt.float16
    f32 = mybir.dt.float32
    d_in = nc.dram_tensor("d_in", [B_LOC, P, FREE], f16, kind="ExternalInput")
    t_out = nc.dram_tensor("t_out", [B_LOC, P, FREE], f16, kind="ExternalOutput")

    with tile.TileContext(nc) as tc, ExitStack() as ctx:
        singles = ctx.enter_context(tc.tile_pool(name="singles", bufs=1))
        in_pool = ctx.enter_context(tc.tile_pool(name="in", bufs=5))
        w_pool = ctx.enter_context(tc.tile_pool(name="w", bufs=4))
        csc_pool = ctx.enter_context(tc.tile_pool(name="csc", bufs=3))
        small = ctx.enter_context(tc.tile_pool(name="small", bufs=8))
        psum = ctx.enter_context(tc.tile_pool(name="psum", bufs=2, space="PSUM"))

        # All-ones/1024 weight (2^-10, exact in fp16): one matmul with this
        # lhsT broadcasts column sums/1024 of its rhs to all 128 partitions.
        ones_k = singles.tile([P, P], f16)
        nc.vector.memset(ones_k[:], 1.0 / 1024.0)

        # Tile start offsets (in matrices).
        starts = []
        m0 = 0
        for sz in TILE_SIZES:
            starts.append(m0)
            m0 += sz

        def front(t):
            """Emit load + colsum matmuls + csc copy for tile t.

            Emitted one tile ahead of back(t-1) so csc(t) lands in ACT's
            queue before pass B of the previous tile -- otherwise DVE
            starves waiting for csc at each tile boundary.
            """
            sz, m0 = TILE_SIZES[t], starts[t]
            in_t = in_pool.tile([P, MAXSZ * FREE], f16)
            for m in range(sz):
                nc.sync.dma_start(out=in_t[:, m * FREE:(m + 1) * FREE],
                                  in_=d_in[m0 + m])
            # csc0 = colsum/1024 per matrix, accumulated over the 4 row
            # chunks into one PSUM bank per matrix (FD=512 f32 = 1 bank).
            pt = psum.tile([P, MAXSZ * N], f32)
            for m in range(sz):
                for c in range(CHUNKS):
                    k = m * CHUNKS + c
                    nc.tensor.matmul(
                        out=pt[:, m * N:(m + 1) * N],
                        lhsT=ones_k[:],
                        rhs=in_t[:, k * N:(k + 1) * N],
                        start=(c == 0),
                        stop=(c == CHUNKS - 1),
                    )
            # PSUM -> SBUF fp16, whole tile in one ACT instruction.
            csc = csc_pool.tile([P, MAXSZ * N], f16)
            nc.scalar.activation(out=csc[:, :sz * N], in_=pt[:, :sz * N],
                                 func=mybir.ActivationFunctionType.Copy,
                                 bias=0.0, scale=1.0)
            return in_t, csc

        def back(t, in_t, csc):
            """Emit pass A (stt+accum), a', pass B, store for tile t."""
            sz, m0 = TILE_SIZES[t], starts[t]
            kch = sz * CHUNKS
            # Pass A: w = -0.5*D + csc0 (col-centered, scaled);
            # accum a[:,k] = rowsum(w chunk) -- absorbs the global term.
            w = w_pool.tile([P, MAXSZ * FREE], f16)
            a = small.tile([P, MAXSZ * CHUNKS], f32)
            for m in range(sz):
                for c in range(CHUNKS):
                    k = m * CHUNKS + c
                    sl = slice(k * N, (k + 1) * N)
                    nc.vector.scalar_tensor_tensor(
                        out=w[:, sl], in0=in_t[:, sl], scalar=-0.5,
                        in1=csc[:, m * N:(m + 1) * N],
                        op0=mybir.AluOpType.mult, op1=mybir.AluOpType.add,
                        accum_out=a[:, k:k + 1],
                    )

            # a' = -a/512 = -(row mean of w); tiny.
            ap_t = small.tile([P, MAXSZ * CHUNKS], f32)
            nc.scalar.activation(out=ap_t[:, :kch], in_=a[:, :kch],
                                 func=mybir.ActivationFunctionType.Copy,
                                 bias=0.0, scale=-1.0 / 512.0)

            # Pass B: T = w + a'[p, k], in place.
            n_dve = PASS_B_DVE_CHUNKS[t]
            for k in range(kch):
                sl = slice(k * N, (k + 1) * N)
                if k >= kch - n_dve:
                    nc.vector.tensor_scalar(
                        out=w[:, sl], in0=w[:, sl],
                        scalar1=ap_t[:, k:k + 1], scalar2=None,
                        op0=mybir.AluOpType.add)
                else:
                    nc.scalar.activation(
                        out=w[:, sl], in_=w[:, sl],
                        func=mybir.ActivationFunctionType.Identity,
                        bias=ap_t[:, k:k + 1], scale=1.0)

            for m in range(sz):
                nc.gpsimd.dma_start(out=t_out[m0 + m],
                                    in_=w[:, m * FREE:(m + 1) * FREE])

        # Software-pipelined emission: front(t+1), front(t+2) before back(t)
        # so csc lands in ACT's queue two tiles ahead of pass B.
        T_TILES = len(TILE_SIZES)
        pend = [front(0), front(1)]
        for t in range(T_TILES):
            if t + 2 < T_TILES:
                pend.append(front(t + 2))
            back(t, *pend.pop(0))

    nc.compile()
    return nc


def _get_nc():
    global _COMPILED
    if _COMPILED is None:
        _COMPILED = _build()
    return _COMPILED


def kernel(D: np.ndarray) -> np.ndarray:
    global LAST_RESULTS
    D = np.asarray(D)
    assert D.shape == (B, N, N), D.shape
    Dh = D.astype(np.float16)
    # per-matrix block layout [128, 2048]: partition p holds rows 4p..4p+3.
    shards = Dh.reshape(N_CORES, B_LOC, P, FREE)
    nc = _get_nc()
    in_maps = [{"d_in": np.ascontiguousarray(shards[i])} for i in range(N_CORES)]
    res = run_bass_kernel_spmd(nc, in_maps, core_ids=list(range(N_CORES)))
    LAST_RESULTS = res
    out = np.stack([res.results[i]["t_out"] for i in range(N_CORES)])
    return np.ascontiguousarray(out).reshape(B, N, N).astype(np.float32)
